# revision 9
# baseline (speedup 1.0000x reference)
"""GroupWiseLinear Trainium2 kernel.

out[b, c] = dot(W[0, c, :], x[b, group_of[c], :]) + bias[0, c], then a final
class-permutation gather, for two independent branches (co / cl).

Sharding: the 128 ragged group-segments (64 per branch) are chopped into
pieces of <= T classes and LPT-assigned across all 8 cores (cores freely mix
branches; the host unshard step composes the final permutation and adds the
bias, so class placement is arbitrary).  Every core runs the SAME program over
S slots whose widths come from a shared profile = elementwise max of each
core's sorted piece widths (rounded up to GRAN), so the instruction stream is
SPMD-uniform while W columns are only padded by the profile slack.

Device pipeline per core (bf16):
  - xw: [128, NXW]  chunked slabs, each = per-slot x^T stationaries
        ([128, 64] per (slot, k-chunk)) followed by the chunk's W^T columns.
        One DMA per chunk, all on the SP queue so transfers stay in slot
        order; chunk sizes taper at the end (last chunk = 1 narrow slot) so
        almost no compute remains after the final byte lands.
  - PSUM banks are aligned to chunk boundaries (<=512 cols each, never
        shared across chunks) so the DVE bank copies never block later
        matmuls via tile-level write-after-read hazards.
  - o:  [64, totW]  bf16, written by three DMAs: bulk (chunks 0..n-3),
        then chunk n-2, then the tiny final chunk; bias is added on host.
"""

import ml_dtypes
import numpy as np

import concourse.bacc as bacc
import concourse.tile as tile
from concourse import mybir
from concourse.bass_utils import run_bass_kernel_spmd

B = 64          # batch
H = 512         # hidden
G = 64          # groups per branch
KC = H // 128   # contraction chunks
NCORES = 8
CAP = 1024      # class columns per core (2 * 4096 / 8)
T = 256         # max piece width (chop granularity)
GRAN = 16       # slot width granularity
PSUM_COLS = 512
NBANKS = 8

_cache = {}


def _segments(go):
    """Runs of equal group id -> list of (group, class_start, length)."""
    go = np.asarray(go).astype(np.int64)
    segs = []
    n = len(go)
    i = 0
    while i < n:
        g = int(go[i])
        j = i
        while j < n and go[j] == g:
            j += 1
        segs.append((g, i, j - i))
        i = j
    return segs


def _plan(co_group_of, cl_group_of):
    """Chop segments, LPT-assign pieces to cores, build the shared profile."""
    pieces = []
    for b, go in ((0, co_group_of), (1, cl_group_of)):
        for g, st, L in _segments(go):
            off = 0
            while L > 0:
                w = min(T, L)
                pieces.append((b, g, st + off, w))
                off += w
                L -= w
    pieces.sort(key=lambda p: (-p[3], p[0], p[2]))
    loads = [0] * NCORES
    assign = [[] for _ in range(NCORES)]
    for p in pieces:
        c = min(range(NCORES), key=lambda k: (loads[k] + p[3] > CAP, loads[k], k))
        w = p[3]
        if loads[c] + w > CAP:
            room = CAP - loads[c]
            if room > 0:
                assign[c].append((p[0], p[1], p[2], room))
                loads[c] += room
                p = (p[0], p[1], p[2] + room, w - room)
            c = min(range(NCORES), key=lambda k: (loads[k], k))
        assign[c].append(p)
        loads[c] += p[3]
    for a in assign:
        a.sort(key=lambda p: (-p[3], p[0], p[2]))
    S = max(len(a) for a in assign)
    prof = []
    for i in range(S):
        m = max((a[i][3] if i < len(a) else 0) for a in assign)
        prof.append(int(-(-m // GRAN) * GRAN))
    return assign, prof


def _layout(prof):
    """Chunking (tapered), chunk-aligned PSUM banks, column offsets."""
    S = len(prof)
    goff = [0]
    for w in prof:
        goff.append(goff[-1] + w)
    totW = goff[-1]

    bps = [(64 + w) * 1024 for w in prof]  # slab bytes per slot

    # Tapered chunks: last chunk = 1 slot, second-to-last = next 2 slots,
    # the rest split into ~equal-byte chunks of <=512 cols each.
    chunks = []
    tail1 = [(S - 1, S)] if S >= 1 else []
    t2lo = max(0, S - 3)
    tail2 = [(t2lo, S - 1)] if S >= 2 and t2lo < S - 1 else []
    head_end = t2lo if tail2 else (S - 1 if tail1 else 0)
    if head_end > 0:
        head_bytes = sum(bps[:head_end])
        n_head = max(1, round(head_bytes / (512 * 1024)))
        target = head_bytes / n_head
        lo = 0
        acc = 0
        cols = 0
        for j in range(head_end):
            if cols + prof[j] > PSUM_COLS or (acc >= target and j > lo):
                chunks.append((lo, j))
                lo = j
                acc = 0
                cols = 0
            acc += bps[j]
            cols += prof[j]
        if lo < head_end:
            chunks.append((lo, head_end))
    chunks += tail2 + tail1

    # PSUM banks: within each chunk, split at <=512 cols (chunk-aligned)
    banks = []  # (first_slot, last_slot_excl, base_col)
    slot_bank = []
    for lo, hi in chunks:
        cur = lo
        used = 0
        for j in range(lo, hi):
            if used + prof[j] > PSUM_COLS:
                banks.append((cur, j, goff[cur]))
                cur = j
                used = 0
            slot_bank.append(len(banks))
            used += prof[j]
        banks.append((cur, hi, goff[cur]))
    return goff, totW, slot_bank, banks, chunks


def _program(prof, dt=mybir.dt.bfloat16):
    S = len(prof)
    goff, totW, slot_bank, banks, chunks = _layout(prof)
    nb = len(banks)
    nch = len(chunks)
    nxw = sum((hi - lo) * KC * 64 + KC * (goff[hi] - goff[lo]) for lo, hi in chunks)

    nc = bacc.Bacc("TRN2", target_bir_lowering=False, debug=False, num_devices=8)
    xw_d = nc.dram_tensor("xw", [128, nxw], dt, kind="ExternalInput")
    o_d = nc.dram_tensor("o", [64, totW], dt, kind="ExternalOutput")

    # output DMA split points (column space): bulk | chunk n-2 | chunk n-1
    cutA = goff[chunks[-2][0]] if nch >= 2 else totW
    cutB = goff[chunks[-1][0]]

    with tile.TileContext(nc) as tc:
        with (
            tc.tile_pool(name="sb", bufs=1) as sb,
            tc.tile_pool(name="ps", bufs=1, space="PSUM") as ps,
        ):
            xw_tiles = []
            dbase = 0
            for ci, (lo, hi) in enumerate(chunks):
                cols = goff[hi] - goff[lo]
                ccols = (hi - lo) * KC * 64 + KC * cols
                xw = sb.tile([128, ccols], dt, tag=f"xw{ci}", name=f"xw{ci}")
                nc.sync.dma_start(xw[:], xw_d[:, dbase : dbase + ccols])
                xw_tiles.append((xw, lo, hi, (hi - lo) * KC * 64, cols))
                dbase += ccols

            pbanks = [
                ps.tile([64, PSUM_COLS], mybir.dt.float32, tag=f"pb{i % NBANKS}",
                        name=f"pb{i}", bufs=1)
                for i in range(min(nb, NBANKS))
            ]
            ob = sb.tile([64, totW], dt, tag="ob")

            for ci, (xw, lo, hi, woff, cols) in enumerate(xw_tiles):
                for j in range(lo, hi):
                    w = prof[j]
                    bi = slot_bank[j]
                    bfirst, blast, bbase = banks[bi]
                    po = goff[j] - bbase
                    loc = goff[j] - goff[lo]
                    acc = pbanks[bi % NBANKS]
                    for k in range(KC):
                        nc.tensor.matmul(
                            acc[0:64, po : po + w],
                            xw[:, ((j - lo) * KC + k) * 64 : ((j - lo) * KC + k + 1) * 64],
                            xw[:, woff + k * cols + loc : woff + k * cols + loc + w],
                            start=(k == 0),
                            stop=(k == KC - 1),
                        )
                    if j == blast - 1:  # bank complete -> convert to bf16
                        bw = goff[blast] - bbase
                        nc.vector.tensor_copy(
                            ob[0:64, bbase : bbase + bw], acc[0:64, 0:bw]
                        )
                        if goff[j] + w == cutA and cutA > 0:
                            nc.scalar.dma_start(o_d[:, 0:cutA], ob[0:64, 0:cutA])
                        if goff[j] + w == cutB and cutB > cutA:
                            nc.scalar.dma_start(
                                o_d[:, cutA:cutB], ob[0:64, cutA:cutB]
                            )
            nc.sync.dma_start(o_d[:, cutB:totW], ob[0:64, cutB:totW])

    nc.compile()
    return nc


def _host_prep(x, Ws, pieces, prof):
    """Build xw for one core.  pieces: list of (branch, group, cls0, w)."""
    goff, totW, slot_bank, banks, chunks = _layout(prof)
    nxw = sum((hi - lo) * KC * 64 + KC * (goff[hi] - goff[lo]) for lo, hi in chunks)
    xw = np.zeros((128, nxw), ml_dtypes.bfloat16)
    dbase = 0
    for lo, hi in chunks:
        cols = goff[hi] - goff[lo]
        woff = dbase + (hi - lo) * KC * 64
        for j in range(lo, min(hi, len(pieces))):
            b, g, cls0, wr = pieces[j]
            xs = x[:, b * G + g, :].reshape(B, KC, 128).transpose(2, 1, 0).reshape(128, KC * 64)
            xw[:, dbase + (j - lo) * KC * 64 : dbase + (j - lo + 1) * KC * 64] = xs
            wseg = Ws[b][cls0 : cls0 + wr, :].reshape(wr, KC, 128).transpose(2, 1, 0)
            loc = goff[j] - goff[lo]
            for k in range(KC):
                xw[:, woff + k * cols + loc : woff + k * cols + loc + wr] = wseg[:, k, :]
        dbase += (hi - lo) * KC * 64 + KC * cols
    return {"xw": xw}


def kernel(x, co_W, cl_W, co_b, cl_b, co_group_of, cl_group_of, co_index,
           cl_index, group_len, _return_raw=False):
    x = np.asarray(x, np.float32)
    assign, prof = _plan(co_group_of, cl_group_of)
    goff, totW, slot_bank, banks, chunks = _layout(prof)

    key = ("v5", tuple(prof))
    if key not in _cache:
        _cache.clear()
        _cache[key] = _program(prof)
    nc = _cache[key]

    Ws = (np.asarray(co_W, np.float32)[0], np.asarray(cl_W, np.float32)[0])
    bs = (np.asarray(co_b, np.float32)[0], np.asarray(cl_b, np.float32)[0])
    in_maps = [_host_prep(x, Ws, assign[c], prof) for c in range(NCORES)]

    res = run_bass_kernel_spmd(nc, in_maps, list(range(NCORES)))

    NC_CLS = len(np.asarray(co_group_of))
    fulls = [np.empty((B, NC_CLS), np.float32) for _ in range(2)]
    for c in range(NCORES):
        o = np.asarray(res.results[c]["o"], ml_dtypes.bfloat16).astype(np.float32)
        for j, (b, g, cls0, wr) in enumerate(assign[c]):
            fulls[b][:, cls0 : cls0 + wr] = o[:, goff[j] : goff[j] + wr]
    fulls[0] += bs[0][None, :]
    fulls[1] += bs[1][None, :]
    co_out = fulls[0][:, np.asarray(co_index).astype(np.int64)]
    cl_out = fulls[1][:, np.asarray(cl_index).astype(np.int64)]
    return co_out, cl_out


# revision 14
# speedup vs baseline: 1.0740x; 1.0740x over previous
"""GroupWiseLinear Trainium2 kernel.

out[b, c] = dot(W[0, c, :], x[b, group_of[c], :]) + bias[0, c], then a final
class-permutation gather, for two independent branches (co / cl).

Sharding: the 128 ragged group-segments (64 per branch) are chopped into
pieces of <= T classes and LPT-assigned across all 8 cores (cores freely mix
branches; the host unshard step composes the final permutation and adds the
bias, so class placement is arbitrary).  Every core runs the SAME program over
S slots whose widths come from a shared profile = elementwise max of each
core's sorted piece widths (rounded up to GRAN), so the instruction stream is
SPMD-uniform while W columns are only padded by the profile slack.

Device pipeline per core (bf16):
  - xw: [128, NXW]  chunked slabs, each = per-slot x^T stationaries
        ([128, 64] per (slot, k-chunk)) followed by the chunk's W^T columns.
        One DMA per chunk, all on the SP queue so transfers stay in slot
        order; chunk sizes taper at the end (last chunk = 1 narrow slot) so
        almost no compute remains after the final byte lands.
  - PSUM banks are aligned to chunk boundaries (<=512 cols each, never
        shared across chunks) so the DVE bank copies never block later
        matmuls via tile-level write-after-read hazards.
  - o:  [64, totW]  bf16, written by three DMAs: bulk (chunks 0..n-3),
        then chunk n-2, then the tiny final chunk; bias is added on host.
"""

import ml_dtypes
import numpy as np

import concourse.bacc as bacc
import concourse.tile as tile
from concourse import mybir
from concourse.bass_utils import run_bass_kernel_spmd

B = 64          # batch
H = 512         # hidden
G = 64          # groups per branch
KC = H // 128   # contraction chunks
NCORES = 8
CAP = 1024      # class columns per core (2 * 4096 / 8)
T = 256         # max piece width (chop granularity)
GRAN = 16       # slot width granularity
PSUM_COLS = 512
NBANKS = 8

_cache = {}


def _segments(go):
    """Runs of equal group id -> list of (group, class_start, length)."""
    go = np.asarray(go).astype(np.int64)
    segs = []
    n = len(go)
    i = 0
    while i < n:
        g = int(go[i])
        j = i
        while j < n and go[j] == g:
            j += 1
        segs.append((g, i, j - i))
        i = j
    return segs


def _plan(co_group_of, cl_group_of):
    """Chop segments, LPT-assign pieces to cores, build the shared profile."""
    pieces = []
    for b, go in ((0, co_group_of), (1, cl_group_of)):
        for g, st, L in _segments(go):
            off = 0
            while L > 0:
                w = min(T, L)
                pieces.append((b, g, st + off, w))
                off += w
                L -= w
    pieces.sort(key=lambda p: (-p[3], p[0], p[2]))
    loads = [0] * NCORES
    assign = [[] for _ in range(NCORES)]
    for p in pieces:
        c = min(range(NCORES), key=lambda k: (loads[k] + p[3] > CAP, loads[k], k))
        w = p[3]
        if loads[c] + w > CAP:
            room = CAP - loads[c]
            if room > 0:
                assign[c].append((p[0], p[1], p[2], room))
                loads[c] += room
                p = (p[0], p[1], p[2] + room, w - room)
            c = min(range(NCORES), key=lambda k: (loads[k], k))
        assign[c].append(p)
        loads[c] += p[3]
    for a in assign:
        a.sort(key=lambda p: (-p[3], p[0], p[2]))
    S = max(len(a) for a in assign)
    prof = []
    for i in range(S):
        m = max((a[i][3] if i < len(a) else 0) for a in assign)
        prof.append(int(-(-m // GRAN) * GRAN))
    return assign, prof


def _layout(prof):
    """Chunking (tapered), chunk-aligned PSUM banks, column offsets."""
    S = len(prof)
    goff = [0]
    for w in prof:
        goff.append(goff[-1] + w)
    totW = goff[-1]

    bps = [(64 + w) * 1024 for w in prof]  # slab bytes per slot

    # Tapered chunks: last chunk = 1 slot, second-to-last = next 2 slots,
    # the rest split into ~equal-byte chunks of <=512 cols each.
    chunks = []
    tail1 = [(S - 1, S)] if S >= 1 else []
    t2lo = max(0, S - 3)
    tail2 = [(t2lo, S - 1)] if S >= 2 and t2lo < S - 1 else []
    head_end = t2lo if tail2 else (S - 1 if tail1 else 0)
    if head_end > 0:
        head_bytes = sum(bps[:head_end])
        n_head = max(1, round(head_bytes / (512 * 1024)))
        target = head_bytes / n_head
        lo = 0
        acc = 0
        cols = 0
        for j in range(head_end):
            if cols + prof[j] > PSUM_COLS or (acc >= target and j > lo):
                chunks.append((lo, j))
                lo = j
                acc = 0
                cols = 0
            acc += bps[j]
            cols += prof[j]
        if lo < head_end:
            chunks.append((lo, head_end))
    chunks += tail2 + tail1

    # PSUM banks: within each chunk, split at <=512 cols (chunk-aligned)
    banks = []  # (first_slot, last_slot_excl, base_col)
    slot_bank = []
    for lo, hi in chunks:
        cur = lo
        used = 0
        for j in range(lo, hi):
            if used + prof[j] > PSUM_COLS:
                banks.append((cur, j, goff[cur]))
                cur = j
                used = 0
            slot_bank.append(len(banks))
            used += prof[j]
        banks.append((cur, hi, goff[cur]))
    return goff, totW, slot_bank, banks, chunks


def _program(prof, dt=mybir.dt.bfloat16):
    S = len(prof)
    goff, totW, slot_bank, banks, chunks = _layout(prof)
    nb = len(banks)
    nch = len(chunks)
    nxw = sum((hi - lo) * KC * 64 + KC * (goff[hi] - goff[lo]) for lo, hi in chunks)

    nc = bacc.Bacc("TRN2", target_bir_lowering=False, debug=False, num_devices=8)
    xw_d = nc.dram_tensor("xw", [128, nxw], dt, kind="ExternalInput")
    o_d = nc.dram_tensor("o", [64, totW], dt, kind="ExternalOutput")

    # output DMA split: bulk [0, cut) on Act fires a few chunks early; the
    # tail [cut, totW) (~>=128 cols) goes last on SP with minimal latency
    cut = totW
    for lo, hi in reversed(chunks):
        if totW - goff[lo] >= 128:
            cut = goff[lo]
            break
    if cut == totW:
        cut = 0

    with tile.TileContext(nc) as tc:
        with (
            tc.tile_pool(name="sb", bufs=1) as sb,
            tc.tile_pool(name="ps", bufs=1, space="PSUM") as ps,
        ):
            xw_tiles = []
            dbase = 0
            for ci, (lo, hi) in enumerate(chunks):
                cols = goff[hi] - goff[lo]
                ccols = (hi - lo) * KC * 64 + KC * cols
                xw = sb.tile([128, ccols], dt, tag=f"xw{ci}", name=f"xw{ci}")
                nc.sync.dma_start(xw[:], xw_d[:, dbase : dbase + ccols])
                xw_tiles.append((xw, lo, hi, (hi - lo) * KC * 64, cols))
                dbase += ccols

            pbanks = [
                ps.tile([64, PSUM_COLS], mybir.dt.float32, tag=f"pb{i % NBANKS}",
                        name=f"pb{i}", bufs=1)
                for i in range(min(nb, NBANKS))
            ]
            ob = sb.tile([64, totW], dt, tag="ob")

            for ci, (xw, lo, hi, woff, cols) in enumerate(xw_tiles):
                for j in range(lo, hi):
                    w = prof[j]
                    bi = slot_bank[j]
                    bfirst, blast, bbase = banks[bi]
                    po = goff[j] - bbase
                    loc = goff[j] - goff[lo]
                    acc = pbanks[bi % NBANKS]
                    for k in range(KC):
                        nc.tensor.matmul(
                            acc[0:64, po : po + w],
                            xw[:, ((j - lo) * KC + k) * 64 : ((j - lo) * KC + k + 1) * 64],
                            xw[:, woff + k * cols + loc : woff + k * cols + loc + w],
                            start=(k == 0),
                            stop=(k == KC - 1),
                        )
                    if j == blast - 1:  # bank complete -> convert to bf16
                        bw = goff[blast] - bbase
                        nc.vector.tensor_copy(
                            ob[0:64, bbase : bbase + bw], acc[0:64, 0:bw]
                        )
                        if goff[j] + w == cut and cut > 0:
                            nc.scalar.dma_start(o_d[:, 0:cut], ob[0:64, 0:cut])
            nc.sync.dma_start(o_d[:, cut:totW], ob[0:64, cut:totW])

    nc.compile()
    return nc


def _host_prep(x, Ws, pieces, prof):
    """Build xw for one core.  pieces: list of (branch, group, cls0, w)."""
    goff, totW, slot_bank, banks, chunks = _layout(prof)
    nxw = sum((hi - lo) * KC * 64 + KC * (goff[hi] - goff[lo]) for lo, hi in chunks)
    xw = np.zeros((128, nxw), ml_dtypes.bfloat16)
    dbase = 0
    for lo, hi in chunks:
        cols = goff[hi] - goff[lo]
        woff = dbase + (hi - lo) * KC * 64
        for j in range(lo, min(hi, len(pieces))):
            b, g, cls0, wr = pieces[j]
            xs = x[:, b * G + g, :].reshape(B, KC, 128).transpose(2, 1, 0).reshape(128, KC * 64)
            xw[:, dbase + (j - lo) * KC * 64 : dbase + (j - lo + 1) * KC * 64] = xs
            wseg = Ws[b][cls0 : cls0 + wr, :].reshape(wr, KC, 128).transpose(2, 1, 0)
            loc = goff[j] - goff[lo]
            for k in range(KC):
                xw[:, woff + k * cols + loc : woff + k * cols + loc + wr] = wseg[:, k, :]
        dbase += (hi - lo) * KC * 64 + KC * cols
    return {"xw": xw}


def kernel(x, co_W, cl_W, co_b, cl_b, co_group_of, cl_group_of, co_index,
           cl_index, group_len, _return_raw=False):
    x = np.asarray(x, np.float32)
    assign, prof = _plan(co_group_of, cl_group_of)
    goff, totW, slot_bank, banks, chunks = _layout(prof)

    key = ("v5", tuple(prof))
    if key not in _cache:
        _cache.clear()
        _cache[key] = _program(prof)
    nc = _cache[key]

    Ws = (np.asarray(co_W, np.float32)[0], np.asarray(cl_W, np.float32)[0])
    bs = (np.asarray(co_b, np.float32)[0], np.asarray(cl_b, np.float32)[0])
    in_maps = [_host_prep(x, Ws, assign[c], prof) for c in range(NCORES)]

    res = run_bass_kernel_spmd(nc, in_maps, list(range(NCORES)))

    NC_CLS = len(np.asarray(co_group_of))
    fulls = [np.empty((B, NC_CLS), np.float32) for _ in range(2)]
    for c in range(NCORES):
        o = np.asarray(res.results[c]["o"], ml_dtypes.bfloat16).astype(np.float32)
        for j, (b, g, cls0, wr) in enumerate(assign[c]):
            fulls[b][:, cls0 : cls0 + wr] = o[:, goff[j] : goff[j] + wr]
    fulls[0] += bs[0][None, :]
    fulls[1] += bs[1][None, :]
    co_out = fulls[0][:, np.asarray(co_index).astype(np.int64)]
    cl_out = fulls[1][:, np.asarray(cl_index).astype(np.int64)]
    return co_out, cl_out


# revision 17
# speedup vs baseline: 1.1335x; 1.0554x over previous
"""GroupWiseLinear Trainium2 kernel (hand-rolled sync, no TileContext).

out[b, c] = dot(W[0, c, :], x[b, group_of[c], :]) + bias[0, c], then a final
class-permutation gather, for two independent branches (co / cl).

Sharding: the 128 ragged group-segments (64 per branch) are chopped into
pieces of <= T classes and LPT-assigned across all 8 cores (cores freely mix
branches; the host unshard step composes the final permutation and adds the
bias, so class placement is arbitrary).  Every core runs the SAME program over
S slots whose widths come from a shared profile = elementwise max of each
core's sorted piece widths (rounded up to GRAN), so the instruction stream is
SPMD-uniform while W columns are only padded by the profile slack.

Device pipeline per core (bf16), synchronized with five manual semaphores
(no Tile framework: no init barrier, no epilogue barrier cascade):
  SP  : one slab DMA per chunk ([slot x^T stationaries | W^T columns], slot
        order, tapered sizes) each bumping s_in by 16; then the small tail
        output DMA after the last bank copy.
  PE  : per chunk, wait s_in then 4 accumulating matmuls per slot
        (x stationary [128,64], W moving [128,w]) into a chunk-aligned PSUM
        bank region; the bank's last matmul bumps s_pe.
  DVE : per bank, wait s_pe then convert f32->bf16 into the output buffer,
        bumping s_cp.
  Act : the bulk output DMA [0, cut) once its banks are copied.
"""

import ml_dtypes
import numpy as np

import concourse.bacc as bacc
from concourse import mybir
from concourse.bass_utils import run_bass_kernel_spmd

B = 64          # batch
H = 512         # hidden
G = 64          # groups per branch
KC = H // 128   # contraction chunks
NCORES = 8
CAP = 1024      # class columns per core (2 * 4096 / 8)
T = 256         # max piece width (chop granularity)
GRAN = 16       # slot width granularity
PSUM_COLS = 512

_cache = {}


def _segments(go):
    """Runs of equal group id -> list of (group, class_start, length)."""
    go = np.asarray(go).astype(np.int64)
    segs = []
    n = len(go)
    i = 0
    while i < n:
        g = int(go[i])
        j = i
        while j < n and go[j] == g:
            j += 1
        segs.append((g, i, j - i))
        i = j
    return segs


def _plan(co_group_of, cl_group_of):
    """Chop segments, LPT-assign pieces to cores, build the shared profile."""
    pieces = []
    for b, go in ((0, co_group_of), (1, cl_group_of)):
        for g, st, L in _segments(go):
            off = 0
            while L > 0:
                w = min(T, L)
                pieces.append((b, g, st + off, w))
                off += w
                L -= w
    pieces.sort(key=lambda p: (-p[3], p[0], p[2]))
    loads = [0] * NCORES
    assign = [[] for _ in range(NCORES)]
    for p in pieces:
        c = min(range(NCORES), key=lambda k: (loads[k] + p[3] > CAP, loads[k], k))
        w = p[3]
        if loads[c] + w > CAP:
            room = CAP - loads[c]
            if room > 0:
                assign[c].append((p[0], p[1], p[2], room))
                loads[c] += room
                p = (p[0], p[1], p[2] + room, w - room)
            c = min(range(NCORES), key=lambda k: (loads[k], k))
        assign[c].append(p)
        loads[c] += p[3]
    for a in assign:
        a.sort(key=lambda p: (-p[3], p[0], p[2]))
    S = max(len(a) for a in assign)
    prof = []
    for i in range(S):
        m = max((a[i][3] if i < len(a) else 0) for a in assign)
        prof.append(int(-(-m // GRAN) * GRAN))
    return assign, prof


def _layout(prof):
    """Chunking (tapered), chunk-aligned PSUM banks, column offsets."""
    S = len(prof)
    goff = [0]
    for w in prof:
        goff.append(goff[-1] + w)
    totW = goff[-1]

    bps = [(64 + w) * 1024 for w in prof]  # slab bytes per slot

    # Tapered chunks: last chunk = 1 slot, second-to-last = next 2 slots,
    # the rest split into ~equal-byte chunks of <=512 cols each.
    chunks = []
    tail1 = [(S - 1, S)] if S >= 1 else []
    t2lo = max(0, S - 3)
    tail2 = [(t2lo, S - 1)] if S >= 2 and t2lo < S - 1 else []
    head_end = t2lo if tail2 else (S - 1 if tail1 else 0)
    if head_end > 0:
        head_bytes = sum(bps[:head_end])
        n_head = max(1, round(head_bytes / (512 * 1024)))
        target = head_bytes / n_head
        lo = 0
        acc = 0
        cols = 0
        for j in range(head_end):
            if cols + prof[j] > PSUM_COLS or (acc >= target and j > lo):
                chunks.append((lo, j))
                lo = j
                acc = 0
                cols = 0
            acc += bps[j]
            cols += prof[j]
        if lo < head_end:
            chunks.append((lo, head_end))
    chunks += tail2 + tail1

    # PSUM banks: within each chunk, split at <=512 cols (chunk-aligned)
    banks = []  # (first_slot, last_slot_excl, base_col)
    slot_bank = []
    for lo, hi in chunks:
        cur = lo
        used = 0
        for j in range(lo, hi):
            if used + prof[j] > PSUM_COLS:
                banks.append((cur, j, goff[cur]))
                cur = j
                used = 0
            slot_bank.append(len(banks))
            used += prof[j]
        banks.append((cur, hi, goff[cur]))

    # bulk/tail output split at a chunk boundary leaving >=128 tail cols
    cut = totW
    for lo, hi in reversed(chunks):
        if totW - goff[lo] >= 128:
            cut = goff[lo]
            break
    if cut == totW:
        cut = 0
    return goff, totW, slot_bank, banks, chunks, cut


def _program(prof, dt=mybir.dt.bfloat16):
    S = len(prof)
    goff, totW, slot_bank, banks, chunks, cut = _layout(prof)
    nb = len(banks)
    assert nb <= 8, nb
    nxw = sum((hi - lo) * KC * 64 + KC * (goff[hi] - goff[lo]) for lo, hi in chunks)

    nc = bacc.Bacc("TRN2", target_bir_lowering=False, debug=False, num_devices=8)
    xw_d = nc.dram_tensor("xw", [128, nxw], dt, kind="ExternalInput")
    o_d = nc.dram_tensor("o", [64, totW], dt, kind="ExternalOutput")

    s_ins = [nc.alloc_semaphore(f"s_in{ci}") for ci in range(len(chunks))]
    s_pe = nc.alloc_semaphore("s_pe")
    s_cp = nc.alloc_semaphore("s_cp")
    s_oa = nc.alloc_semaphore("s_oa")
    s_ob = nc.alloc_semaphore("s_ob")

    xw_sb = []
    dbase = 0
    for ci, (lo, hi) in enumerate(chunks):
        cols = goff[hi] - goff[lo]
        ccols = (hi - lo) * KC * 64 + KC * cols
        xw_sb.append((nc.alloc_sbuf_tensor(f"xw{ci}", [128, ccols], dt),
                      lo, hi, (hi - lo) * KC * 64, cols, dbase, ccols))
        dbase += ccols
    ob = nc.alloc_sbuf_tensor("ob", [64, totW], dt)
    ps = [nc.alloc_psum_tensor(f"pb{i}", [64, PSUM_COLS], mybir.dt.float32)
          for i in range(nb)]

    # SP: input slabs in slot order, one completion semaphore each
    for ci, (xw, lo, hi, woff, cols, db, ccols) in enumerate(xw_sb):
        nc.sync.dma_start(xw[:, :], xw_d[:, db : db + ccols]).then_inc(s_ins[ci], 16)

    # PE: matmuls, gated per chunk
    for ci, (xw, lo, hi, woff, cols, db, ccols) in enumerate(xw_sb):
        nc.tensor.wait_ge(s_ins[ci], 16)
        for j in range(lo, hi):
            w = prof[j]
            bi = slot_bank[j]
            bfirst, blast, bbase = banks[bi]
            po = goff[j] - bbase
            loc = goff[j] - goff[lo]
            acc = ps[bi]
            for k in range(KC):
                mm = nc.tensor.matmul(
                    acc[0:64, po : po + w],
                    xw[:, ((j - lo) * KC + k) * 64 : ((j - lo) * KC + k + 1) * 64],
                    xw[:, woff + k * cols + loc : woff + k * cols + loc + w],
                    start=(k == 0),
                    stop=(k == KC - 1),
                )
                if j == blast - 1 and k == KC - 1:
                    mm.then_inc(s_pe, 1)

    # DVE: bank copies f32 -> bf16
    ncopyA = 0
    for bi, (bfirst, blast, bbase) in enumerate(banks):
        bw = goff[blast] - bbase
        nc.vector.wait_ge(s_pe, bi + 1)
        nc.vector.tensor_copy(ob[0:64, bbase : bbase + bw],
                              ps[bi][0:64, 0:bw]).then_inc(s_cp, 1)
        if goff[blast] <= cut:
            ncopyA = bi + 1

    # Act: bulk output
    if cut > 0:
        nc.scalar.wait_ge(s_cp, ncopyA)
        nc.scalar.dma_start(o_d[:, 0:cut], ob[0:64, 0:cut]).then_inc(s_oa, 16)
        nc.scalar.wait_ge(s_oa, 16)

    # SP: tail output
    nc.sync.wait_ge(s_cp, nb)
    nc.sync.dma_start(o_d[:, cut:totW], ob[0:64, cut:totW]).then_inc(s_ob, 16)
    nc.sync.wait_ge(s_ob, 16)

    nc.compile()
    return nc


def _host_prep(x, Ws, pieces, prof):
    """Build xw for one core.  pieces: list of (branch, group, cls0, w)."""
    goff, totW, slot_bank, banks, chunks, cut = _layout(prof)
    nxw = sum((hi - lo) * KC * 64 + KC * (goff[hi] - goff[lo]) for lo, hi in chunks)
    xw = np.zeros((128, nxw), ml_dtypes.bfloat16)
    dbase = 0
    for lo, hi in chunks:
        cols = goff[hi] - goff[lo]
        woff = dbase + (hi - lo) * KC * 64
        for j in range(lo, min(hi, len(pieces))):
            b, g, cls0, wr = pieces[j]
            xs = x[:, b * G + g, :].reshape(B, KC, 128).transpose(2, 1, 0).reshape(128, KC * 64)
            xw[:, dbase + (j - lo) * KC * 64 : dbase + (j - lo + 1) * KC * 64] = xs
            wseg = Ws[b][cls0 : cls0 + wr, :].reshape(wr, KC, 128).transpose(2, 1, 0)
            loc = goff[j] - goff[lo]
            for k in range(KC):
                xw[:, woff + k * cols + loc : woff + k * cols + loc + wr] = wseg[:, k, :]
        dbase += (hi - lo) * KC * 64 + KC * cols
    return {"xw": xw}


def kernel(x, co_W, cl_W, co_b, cl_b, co_group_of, cl_group_of, co_index,
           cl_index, group_len, _return_raw=False):
    x = np.asarray(x, np.float32)
    assign, prof = _plan(co_group_of, cl_group_of)
    goff, totW, slot_bank, banks, chunks, cut = _layout(prof)

    key = ("v6", tuple(prof))
    if key not in _cache:
        _cache.clear()
        _cache[key] = _program(prof)
    nc = _cache[key]

    Ws = (np.asarray(co_W, np.float32)[0], np.asarray(cl_W, np.float32)[0])
    bs = (np.asarray(co_b, np.float32)[0], np.asarray(cl_b, np.float32)[0])
    in_maps = [_host_prep(x, Ws, assign[c], prof) for c in range(NCORES)]

    res = run_bass_kernel_spmd(nc, in_maps, list(range(NCORES)))

    NC_CLS = len(np.asarray(co_group_of))
    fulls = [np.empty((B, NC_CLS), np.float32) for _ in range(2)]
    for c in range(NCORES):
        o = np.asarray(res.results[c]["o"], ml_dtypes.bfloat16).astype(np.float32)
        for j, (b, g, cls0, wr) in enumerate(assign[c]):
            fulls[b][:, cls0 : cls0 + wr] = o[:, goff[j] : goff[j] + wr]
    fulls[0] += bs[0][None, :]
    fulls[1] += bs[1][None, :]
    co_out = fulls[0][:, np.asarray(co_index).astype(np.int64)]
    cl_out = fulls[1][:, np.asarray(cl_index).astype(np.int64)]
    return co_out, cl_out


# revision 18
# speedup vs baseline: 1.1617x; 1.0249x over previous
"""GroupWiseLinear Trainium2 kernel (hand-rolled sync, no TileContext).

out[b, c] = dot(W[0, c, :], x[b, group_of[c], :]) + bias[0, c], then a final
class-permutation gather, for two independent branches (co / cl).

Sharding: the 128 ragged group-segments (64 per branch) are chopped into
pieces of <= T classes and LPT-assigned across all 8 cores (cores freely mix
branches; the host unshard step composes the final permutation and adds the
bias, so class placement is arbitrary).  Every core runs the SAME program over
S slots whose widths come from a shared profile = elementwise max of each
core's sorted piece widths (rounded up to GRAN), so the instruction stream is
SPMD-uniform while W columns are only padded by the profile slack.

Device pipeline per core (bf16), synchronized with five manual semaphores
(no Tile framework: no init barrier, no epilogue barrier cascade):
  SP  : one slab DMA per chunk ([slot x^T stationaries | W^T columns], slot
        order, tapered sizes) each bumping s_in by 16; then the small tail
        output DMA after the last bank copy.
  PE  : per chunk, wait s_in then 4 accumulating matmuls per slot
        (x stationary [128,64], W moving [128,w]) into a chunk-aligned PSUM
        bank region; the bank's last matmul bumps s_pe.
  DVE : per bank, wait s_pe then convert f32->bf16 into the output buffer,
        bumping s_cp.
  Act : the bulk output DMA [0, cut) once its banks are copied.
"""

import ml_dtypes
import numpy as np

import concourse.bacc as bacc
from concourse import mybir
from concourse.bass_utils import run_bass_kernel_spmd

B = 64          # batch
H = 512         # hidden
G = 64          # groups per branch
KC = H // 128   # contraction chunks
NCORES = 8
CAP = 1024      # class columns per core (2 * 4096 / 8)
T = 256         # max piece width (chop granularity)
GRAN = 4        # slot width granularity
PSUM_COLS = 512

_cache = {}


def _segments(go):
    """Runs of equal group id -> list of (group, class_start, length)."""
    go = np.asarray(go).astype(np.int64)
    segs = []
    n = len(go)
    i = 0
    while i < n:
        g = int(go[i])
        j = i
        while j < n and go[j] == g:
            j += 1
        segs.append((g, i, j - i))
        i = j
    return segs


def _plan(co_group_of, cl_group_of):
    """Chop segments, LPT-assign pieces to cores, build the shared profile."""
    pieces = []
    for b, go in ((0, co_group_of), (1, cl_group_of)):
        for g, st, L in _segments(go):
            off = 0
            while L > 0:
                w = min(T, L)
                pieces.append((b, g, st + off, w))
                off += w
                L -= w
    pieces.sort(key=lambda p: (-p[3], p[0], p[2]))
    loads = [0] * NCORES
    assign = [[] for _ in range(NCORES)]
    for p in pieces:
        c = min(range(NCORES), key=lambda k: (loads[k] + p[3] > CAP, loads[k], k))
        w = p[3]
        if loads[c] + w > CAP:
            room = CAP - loads[c]
            if room > 0:
                assign[c].append((p[0], p[1], p[2], room))
                loads[c] += room
                p = (p[0], p[1], p[2] + room, w - room)
            c = min(range(NCORES), key=lambda k: (loads[k], k))
        assign[c].append(p)
        loads[c] += p[3]
    for a in assign:
        a.sort(key=lambda p: (-p[3], p[0], p[2]))
    S = max(len(a) for a in assign)
    prof = []
    for i in range(S):
        m = max((a[i][3] if i < len(a) else 0) for a in assign)
        prof.append(int(-(-m // GRAN) * GRAN))
    return assign, prof


def _layout(prof):
    """Chunking (tapered), chunk-aligned PSUM banks, column offsets."""
    S = len(prof)
    goff = [0]
    for w in prof:
        goff.append(goff[-1] + w)
    totW = goff[-1]

    bps = [(64 + w) * 1024 for w in prof]  # slab bytes per slot

    # Tapered chunks: last chunk = 1 slot, second-to-last = next 2 slots,
    # the rest split into ~equal-byte chunks of <=512 cols each.
    chunks = []
    tail1 = [(S - 1, S)] if S >= 1 else []
    t2lo = max(0, S - 3)
    tail2 = [(t2lo, S - 1)] if S >= 2 and t2lo < S - 1 else []
    head_end = t2lo if tail2 else (S - 1 if tail1 else 0)
    if head_end > 0:
        head_bytes = sum(bps[:head_end])
        n_head = max(1, round(head_bytes / (512 * 1024)))
        target = head_bytes / n_head
        lo = 0
        acc = 0
        cols = 0
        for j in range(head_end):
            if cols + prof[j] > PSUM_COLS or (acc >= target and j > lo):
                chunks.append((lo, j))
                lo = j
                acc = 0
                cols = 0
            acc += bps[j]
            cols += prof[j]
        if lo < head_end:
            chunks.append((lo, head_end))
    chunks += tail2 + tail1

    # PSUM banks: within each chunk, split at <=512 cols (chunk-aligned)
    banks = []  # (first_slot, last_slot_excl, base_col)
    slot_bank = []
    for lo, hi in chunks:
        cur = lo
        used = 0
        for j in range(lo, hi):
            if used + prof[j] > PSUM_COLS:
                banks.append((cur, j, goff[cur]))
                cur = j
                used = 0
            slot_bank.append(len(banks))
            used += prof[j]
        banks.append((cur, hi, goff[cur]))

    # bulk/tail output split at a chunk boundary leaving >=128 tail cols
    cut = totW
    for lo, hi in reversed(chunks):
        if totW - goff[lo] >= 128:
            cut = goff[lo]
            break
    if cut == totW:
        cut = 0
    return goff, totW, slot_bank, banks, chunks, cut


def _program(prof, dt=mybir.dt.bfloat16):
    S = len(prof)
    goff, totW, slot_bank, banks, chunks, cut = _layout(prof)
    nb = len(banks)
    assert nb <= 8, nb
    nxw = sum((hi - lo) * KC * 64 + KC * (goff[hi] - goff[lo]) for lo, hi in chunks)

    nc = bacc.Bacc("TRN2", target_bir_lowering=False, debug=False, num_devices=8)
    xw_d = nc.dram_tensor("xw", [128, nxw], dt, kind="ExternalInput")
    o_d = nc.dram_tensor("o", [64, totW], dt, kind="ExternalOutput")

    s_ins = [nc.alloc_semaphore(f"s_in{ci}") for ci in range(len(chunks))]
    s_pe = nc.alloc_semaphore("s_pe")
    s_cp = nc.alloc_semaphore("s_cp")
    s_oa = nc.alloc_semaphore("s_oa")
    s_ob = nc.alloc_semaphore("s_ob")

    xw_sb = []
    dbase = 0
    for ci, (lo, hi) in enumerate(chunks):
        cols = goff[hi] - goff[lo]
        ccols = (hi - lo) * KC * 64 + KC * cols
        xw_sb.append((nc.alloc_sbuf_tensor(f"xw{ci}", [128, ccols], dt),
                      lo, hi, (hi - lo) * KC * 64, cols, dbase, ccols))
        dbase += ccols
    ob = nc.alloc_sbuf_tensor("ob", [64, totW], dt)
    ps = [nc.alloc_psum_tensor(f"pb{i}", [64, PSUM_COLS], mybir.dt.float32)
          for i in range(nb)]

    # SP: input slabs in slot order, one completion semaphore each
    for ci, (xw, lo, hi, woff, cols, db, ccols) in enumerate(xw_sb):
        nc.sync.dma_start(xw[:, :], xw_d[:, db : db + ccols]).then_inc(s_ins[ci], 16)

    # PE: matmuls, gated per chunk
    for ci, (xw, lo, hi, woff, cols, db, ccols) in enumerate(xw_sb):
        nc.tensor.wait_ge(s_ins[ci], 16)
        for j in range(lo, hi):
            w = prof[j]
            bi = slot_bank[j]
            bfirst, blast, bbase = banks[bi]
            po = goff[j] - bbase
            loc = goff[j] - goff[lo]
            acc = ps[bi]
            for k in range(KC):
                mm = nc.tensor.matmul(
                    acc[0:64, po : po + w],
                    xw[:, ((j - lo) * KC + k) * 64 : ((j - lo) * KC + k + 1) * 64],
                    xw[:, woff + k * cols + loc : woff + k * cols + loc + w],
                    start=(k == 0),
                    stop=(k == KC - 1),
                )
                if j == blast - 1 and k == KC - 1:
                    mm.then_inc(s_pe, 1)

    # DVE: bank copies f32 -> bf16
    ncopyA = 0
    for bi, (bfirst, blast, bbase) in enumerate(banks):
        bw = goff[blast] - bbase
        nc.vector.wait_ge(s_pe, bi + 1)
        nc.vector.tensor_copy(ob[0:64, bbase : bbase + bw],
                              ps[bi][0:64, 0:bw]).then_inc(s_cp, 1)
        if goff[blast] <= cut:
            ncopyA = bi + 1

    # Act: bulk output
    if cut > 0:
        nc.scalar.wait_ge(s_cp, ncopyA)
        nc.scalar.dma_start(o_d[:, 0:cut], ob[0:64, 0:cut]).then_inc(s_oa, 16)
        nc.scalar.wait_ge(s_oa, 16)

    # SP: tail output
    nc.sync.wait_ge(s_cp, nb)
    nc.sync.dma_start(o_d[:, cut:totW], ob[0:64, cut:totW]).then_inc(s_ob, 16)
    nc.sync.wait_ge(s_ob, 16)

    nc.compile()
    return nc


def _host_prep(x, Ws, pieces, prof):
    """Build xw for one core.  pieces: list of (branch, group, cls0, w)."""
    goff, totW, slot_bank, banks, chunks, cut = _layout(prof)
    nxw = sum((hi - lo) * KC * 64 + KC * (goff[hi] - goff[lo]) for lo, hi in chunks)
    xw = np.zeros((128, nxw), ml_dtypes.bfloat16)
    dbase = 0
    for lo, hi in chunks:
        cols = goff[hi] - goff[lo]
        woff = dbase + (hi - lo) * KC * 64
        for j in range(lo, min(hi, len(pieces))):
            b, g, cls0, wr = pieces[j]
            xs = x[:, b * G + g, :].reshape(B, KC, 128).transpose(2, 1, 0).reshape(128, KC * 64)
            xw[:, dbase + (j - lo) * KC * 64 : dbase + (j - lo + 1) * KC * 64] = xs
            wseg = Ws[b][cls0 : cls0 + wr, :].reshape(wr, KC, 128).transpose(2, 1, 0)
            loc = goff[j] - goff[lo]
            for k in range(KC):
                xw[:, woff + k * cols + loc : woff + k * cols + loc + wr] = wseg[:, k, :]
        dbase += (hi - lo) * KC * 64 + KC * cols
    return {"xw": xw}


def kernel(x, co_W, cl_W, co_b, cl_b, co_group_of, cl_group_of, co_index,
           cl_index, group_len, _return_raw=False):
    x = np.asarray(x, np.float32)
    assign, prof = _plan(co_group_of, cl_group_of)
    goff, totW, slot_bank, banks, chunks, cut = _layout(prof)

    key = ("v6", tuple(prof))
    if key not in _cache:
        _cache.clear()
        _cache[key] = _program(prof)
    nc = _cache[key]

    Ws = (np.asarray(co_W, np.float32)[0], np.asarray(cl_W, np.float32)[0])
    bs = (np.asarray(co_b, np.float32)[0], np.asarray(cl_b, np.float32)[0])
    in_maps = [_host_prep(x, Ws, assign[c], prof) for c in range(NCORES)]

    res = run_bass_kernel_spmd(nc, in_maps, list(range(NCORES)))

    NC_CLS = len(np.asarray(co_group_of))
    fulls = [np.empty((B, NC_CLS), np.float32) for _ in range(2)]
    for c in range(NCORES):
        o = np.asarray(res.results[c]["o"], ml_dtypes.bfloat16).astype(np.float32)
        for j, (b, g, cls0, wr) in enumerate(assign[c]):
            fulls[b][:, cls0 : cls0 + wr] = o[:, goff[j] : goff[j] + wr]
    fulls[0] += bs[0][None, :]
    fulls[1] += bs[1][None, :]
    co_out = fulls[0][:, np.asarray(co_index).astype(np.int64)]
    cl_out = fulls[1][:, np.asarray(cl_index).astype(np.int64)]
    return co_out, cl_out
